# revision 7
# baseline (speedup 1.0000x reference)
"""Self-contained Trainium2 Bass kernel for nn_GAT (GNN message passing).

v2 layout (per core, SPMD across NCORES cores):
  - Graphs LPT-rebalanced across cores (16 graphs/core) to equalize node
    counts; nodes sharded by graph owner core.
  - Within a core nodes are sorted by max(degA, degB) desc and padded to
    slice_pad rows (mult of 128). Self-loops are NOT materialized as edges;
    they are applied locally from per-node a_src/a_dst/h kept in SBUF.
  - Global node table row = m*slice_pad + local_row; rows are
    [h(64 bf16) | a_src(4 f32 as raw bits) | 0...] 256B, rebuilt per layer
    via local matmul + AllGather. Pad rows carry a_src = -300 (poison), so
    junk gather slots contribute exp(leaky(-300+a_dst)) ~= e-60 ~= 0: no
    mask needed. Segment max is skipped entirely (logits are O(1)).
  - Edges sharded by dst owner, laid out node-major: dst local row ->
    (tile, partition); slots padded to uniform L per (group, A|B) where
    A = src table row < T0_ROWS (= cores 0..NCORES/2-1). Junk slots point
    at a poison pad row. Slots within a (dst, lane) run sorted by src row
    (DMA locality).
  - Groups pack consecutive tiles greedily while tig*(L0+L1) <= CCAP.
  - Per layer l+1, slice build (transpose + matmul + table row writes) is
    interleaved into layer l's edge phase per group, so the AllGather can
    start as soon as the last group finishes.
"""

import numpy as np

HID = 16
HEADS = 4
F_HID = HID * HEADS  # 64
NEG_SLOPE = 0.2
P = 128
ROW_F = 128  # table row elems (bf16) = 256B
POISON = -300.0


def build_plan(x, edge_index, batch, ng=128, ncores=8, ccap=64, tigmax=8):
    x = np.asarray(x, np.float32)
    ei = np.asarray(edge_index, np.int64)
    batch = np.asarray(batch, np.int64)
    N, f_in = x.shape
    gpc = ng // ncores

    # ---- graph -> core rebalancing (LPT on node counts, gpc graphs/core) ----
    gcnt = np.bincount(batch, minlength=ng)
    core_of_graph = np.full(ng, -1, np.int64)
    slot_of_graph = np.full(ng, -1, np.int64)
    loads = np.zeros(ncores, np.int64)
    nass = np.zeros(ncores, np.int64)
    for g in np.argsort(-gcnt, kind="stable"):
        cands = np.flatnonzero(nass < gpc)
        m = cands[np.argmin(loads[cands])]
        core_of_graph[g] = m
        slot_of_graph[g] = nass[m]
        nass[m] += 1
        loads[m] += gcnt[g]
    graph_of = np.full((ncores, gpc), -1, np.int64)
    graph_of[core_of_graph, slot_of_graph] = np.arange(ng)

    src = ei[0]
    dst = ei[1]

    node_core = core_of_graph[batch]
    counts = np.bincount(node_core, minlength=ncores)
    slice_pad = int(np.ceil(counts.max() / P) * P)
    if counts.max() == slice_pad:
        slice_pad += P  # ensure at least one pad row exists somewhere
    n_tiles = slice_pad // P
    Npad = ncores * slice_pad
    T0C = ncores // 2
    T0_ROWS = T0C * slice_pad
    assert T0_ROWS < 32768 and (Npad - T0_ROWS) < 32768, (T0_ROWS, Npad)
    core_start = np.concatenate([[0], np.cumsum(counts)])

    src_core = node_core[src]
    isA = src_core < T0C
    degA = np.bincount(dst, weights=isA.astype(np.float64), minlength=N).astype(np.int64)
    degB = np.bincount(dst, weights=(~isA).astype(np.float64), minlength=N).astype(np.int64)

    # within-core permutation: sort by max(degA, degB) desc, then degA+degB
    glob_row = np.empty(N, np.int64)
    row_node = np.full(Npad, -1, np.int64)
    order_n = np.argsort(node_core, kind="stable")
    for m in range(ncores):
        nodes = order_n[core_start[m] : core_start[m + 1]]
        # band sort: max(degA,degB) desc bands of 512, then degA-degB desc
        # inside each band -> tiles are homogeneous in BOTH lanes
        a, b = degA[nodes], degB[nodes]
        o1 = np.argsort(-np.maximum(a, b), kind="stable")
        parts = []
        for i in range(0, len(nodes), 512):
            blk = o1[i : i + 512]
            parts.append(blk[np.argsort(-(a[blk] - b[blk]), kind="stable")])
        nodes = nodes[np.concatenate(parts)]
        glob_row[nodes] = m * slice_pad + np.arange(len(nodes))
        row_node[m * slice_pad + np.arange(len(nodes))] = nodes

    # poison pad rows (one per table half) for junk slots
    mA = int(np.argmin(counts[:T0C]))
    mB = T0C + int(np.argmin(counts[T0C:]))
    assert counts[mA] < slice_pad and counts[mB] < slice_pad
    poisonA_idx = mA * slice_pad + slice_pad - 1
    poisonB_idx = mB * slice_pad + slice_pad - 1 - T0_ROWS
    assert 0 <= poisonA_idx < T0_ROWS and 0 <= poisonB_idx < Npad - T0_ROWS

    # per-tile max degrees across cores
    tile_degA = np.zeros((ncores, n_tiles), np.int64)
    tile_degB = np.zeros((ncores, n_tiles), np.int64)
    for m in range(ncores):
        rows = np.arange(counts[m])
        nodes = row_node[m * slice_pad + rows]
        t = rows // P
        np.maximum.at(tile_degA[m], t, degA[nodes])
        np.maximum.at(tile_degB[m], t, degB[nodes])
    gmaxA = np.maximum(tile_degA.max(axis=0), 1)
    gmaxB = np.maximum(tile_degB.max(axis=0), 1)

    # DP group packing: minimize padded slots + small per-group penalty,
    # subject to tig <= tigmax and tig*(L0+L1) <= ccap
    GPEN = 2  # column-equivalent fixed cost per group (idx DMAs, vector insts)
    INF = 1 << 60
    best = [INF] * (n_tiles + 1)
    best[0] = 0
    choice = [0] * (n_tiles + 1)
    for j in range(1, n_tiles + 1):
        for tig in range(1, min(tigmax, j) + 1):
            i = j - tig
            l0 = int(gmaxA[i:j].max())
            l1 = int(gmaxB[i:j].max())
            if tig * (l0 + l1) > ccap:
                continue
            c = best[i] + tig * (l0 + l1) + GPEN
            if c < best[j]:
                best[j] = c
                choice[j] = tig
    groups = []
    j = n_tiles
    while j > 0:
        tig = choice[j]
        i = j - tig
        groups.append(
            dict(
                base=i,
                tig=tig,
                L0=int(gmaxA[i:j].max()),
                L1=int(gmaxB[i:j].max()),
            )
        )
        j = i
    groups.reverse()

    # offsets in the concatenated idx inputs
    offA = offB = 0
    for g in groups:
        cA, cB = g["tig"] * g["L0"], g["tig"] * g["L1"]
        g["offA"], g["wA"] = offA, cA * 8  # cA*128 idxs / 16 rows
        g["offB"], g["wB"] = offB, cB * 8
        offA += g["wA"]
        offB += g["wB"]
    WA, WB = offA, offB

    # ---- per-core edge slot assignment ----
    dst_owner = node_core[dst]
    loc_row = glob_row[dst] - dst_owner * slice_pad
    src_row = glob_row[src]

    # minor-key src_row: slots within a (dst, lane) run ordered by ascending
    # source table row (DMA descriptor address locality)
    order = np.lexsort((src_row, ~isA, loc_row, dst_owner))
    so_owner = dst_owner[order]
    so_loc = loc_row[order]
    so_isA = isA[order]
    so_srcrow = src_row[order]
    key = so_owner * (Npad * 2) + so_loc * 2 + (~so_isA).astype(np.int64)
    newrun = np.concatenate([[True], key[1:] != key[:-1]])
    run_start = np.flatnonzero(newrun)
    slot = np.arange(len(key)) - run_start[np.cumsum(newrun) - 1]

    tile_of = so_loc // P
    part_of = so_loc % P
    group_of = np.zeros(n_tiles, np.int64)
    for gi, g in enumerate(groups):
        group_of[g["base"] : g["base"] + g["tig"]] = gi

    idxA_cat, idxB_cat = [], []
    for m in range(ncores):
        emask = so_owner == m
        et = tile_of[emask]
        ep = part_of[emask]
        eA = so_isA[emask]
        esrc = so_srcrow[emask]
        eslot = slot[emask]
        eg = group_of[et]

        iA_full = np.full((16, WA), poisonA_idx, np.int16)
        iB_full = np.full((16, WB), poisonB_idx, np.int16)
        for gi, g in enumerate(groups):
            tig, l0, l1 = g["tig"], g["L0"], g["L1"]
            gselA = (eg == gi) & eA
            gselB = (eg == gi) & ~eA
            tt = et - g["base"]
            cidx = tt[gselA] * l0 + eslot[gselA]
            q = cidx * P + ep[gselA]
            iA_full[q % 16, g["offA"] + q // 16] = esrc[gselA].astype(np.int16)
            cidx = tt[gselB] * l1 + eslot[gselB]
            q = cidx * P + ep[gselB]
            iB_full[q % 16, g["offB"] + q // 16] = (esrc[gselB] - T0_ROWS).astype(
                np.int16
            )
            assert esrc[gselA].max(initial=0) < T0_ROWS
            assert (esrc[gselB] - T0_ROWS).max(initial=0) < Npad - T0_ROWS
        idxA_cat.append(np.tile(iA_full, (8, 1)))
        idxB_cat.append(np.tile(iB_full, (8, 1)))

    # ---- xT per core [f_in, slice_pad] ----
    xT = []
    for m in range(ncores):
        xs = np.zeros((slice_pad, f_in), np.float32)
        nodes = row_node[m * slice_pad : m * slice_pad + counts[m]]
        xs[: counts[m]] = x[nodes]
        xT.append(np.ascontiguousarray(xs.T))

    # ---- selectors [128, n_tiles*gpc] per core ----
    sel = []
    for m in range(ncores):
        s = np.zeros((P, n_tiles, gpc), np.float32)
        rows = np.arange(counts[m])
        nodes = row_node[m * slice_pad + rows]
        s[rows % P, rows // P, slot_of_graph[batch[nodes]]] = 1.0
        sel.append(s.reshape(P, n_tiles * gpc))

    # ---- poison [128, n_tiles*4] per core: POISON on pad rows, else 0 ----
    poison = []
    for m in range(ncores):
        pz = np.zeros((P, n_tiles, HEADS), np.float32)
        rows = np.arange(counts[m], slice_pad)
        pz[rows % P, rows // P, :] = POISON
        poison.append(pz.reshape(P, n_tiles * HEADS))

    struct = dict(
        slice_pad=slice_pad,
        n_tiles=n_tiles,
        Npad=Npad,
        T0_ROWS=T0_ROWS,
        groups=groups,
        WA=WA,
        WB=WB,
        f_in=f_in,
        g_per_core=gpc,
        ncls=None,  # fill in later
        ncores=ncores,
        ng=ng,
    )
    glob = dict(
        glob_row=glob_row, row_node=row_node, counts=counts, graph_of=graph_of
    )
    percore = dict(idxA=idxA_cat, idxB=idxB_cat, xT=xT, sel=sel, poison=poison)
    return struct, percore, glob


def augment_weights(W, a_s, a_d):
    W = np.asarray(W, np.float32)
    a_s = np.asarray(a_s, np.float32)
    a_d = np.asarray(a_d, np.float32)
    As = np.zeros((F_HID, HEADS), np.float32)
    Ad = np.zeros((F_HID, HEADS), np.float32)
    for h in range(HEADS):
        As[h * HID : (h + 1) * HID, h] = a_s[h]
        Ad[h * HID : (h + 1) * HID, h] = a_d[h]
    return np.concatenate([W, W @ As, W @ Ad], axis=1).astype(np.float32)


def make_inmaps(inputs, struct, percore, layers=3):
    """Build the per-core input dicts for run_bass_kernel_spmd."""
    ncores = struct["ncores"]
    ws = [
        augment_weights(inputs[f"W{l}"], inputs[f"as{l}"], inputs[f"ad{l}"])
        for l in range(layers)
    ]
    biases = np.concatenate(
        [np.asarray(inputs[f"b{l}"], np.float32) for l in range(layers)]
    )
    bias_rep = np.tile(biases[None, :], (P, 1))
    wr = np.asarray(inputs["Wr"], np.float32)
    br_rep = np.tile(np.asarray(inputs["br"], np.float32)[None, :], (P, 1))
    in_maps = []
    for m in range(ncores):
        im = dict(
            xT=percore["xT"][m],
            idxA=percore["idxA"][m],
            idxB=percore["idxB"][m],
            sel=percore["sel"][m],
            poison=percore["poison"][m],
            biases=bias_rep,
            wr=wr,
            br=br_rep,
        )
        for l in range(layers):
            im[f"w{l}aug"] = ws[l]
        in_maps.append(im)
    return in_maps


def numpy_model(inputs, struct, percore, glob, layers=3):
    """Numpy re-implementation of the device algorithm (same padded layout)."""
    sp = struct["slice_pad"]
    Npad = struct["Npad"]
    T0 = struct["T0_ROWS"]
    ncores = struct["ncores"]
    gpc = struct["g_per_core"]
    nt = struct["n_tiles"]
    ncls = np.asarray(inputs["Wr"]).shape[1]

    ws = [
        augment_weights(inputs[f"W{l}"], inputs[f"as{l}"], inputs[f"ad{l}"])
        for l in range(layers)
    ]
    biases = [np.asarray(inputs[f"b{l}"], np.float32) for l in range(layers)]

    acts = [None] * ncores
    out_logits = np.zeros((struct["ng"], ncls), np.float32)

    def leaky(v):
        return np.where(v >= 0, v, NEG_SLOPE * v).astype(np.float32)

    for layer in range(layers):
        # slice build: h(64) | asrc(4, pads poisoned) | zeros
        table = np.zeros((Npad, ROW_F), np.float32)
        asrc_loc = [None] * ncores
        adst_loc = [None] * ncores
        hloc = [None] * ncores
        for m in range(ncores):
            a = percore["xT"][m].T if layer == 0 else acts[m]
            pr = a @ ws[layer]  # [sp, 72]
            table[m * sp : (m + 1) * sp, :64] = pr[:, :64]
            pz = percore["poison"][m].reshape(P, nt, HEADS)
            pzr = np.transpose(pz, (1, 0, 2)).reshape(sp, HEADS)
            table[m * sp : (m + 1) * sp, 64:68] = pr[:, 64:68] + pzr
            asrc_loc[m] = pr[:, 64:68]
            adst_loc[m] = pr[:, 68:72]
            hloc[m] = pr[:, :64]
        for m in range(ncores):
            e_self = np.exp(leaky(asrc_loc[m] + adst_loc[m]))  # [sp, 4]
            selfmsg = hloc[m].reshape(sp, HEADS, HID) * e_self[:, :, None]
            out = np.zeros((sp, F_HID), np.float32)
            for g in struct["groups"]:
                tig, l0, l1 = g["tig"], g["L0"], g["L1"]
                cA, cB = tig * l0, tig * l1
                C = cA + cB
                iw = percore["idxA"][m][:16, g["offA"] : g["offA"] + g["wA"]]
                iA = iw.T.reshape(-1)[: cA * P].astype(np.int64)
                iw = percore["idxB"][m][:16, g["offB"] : g["offB"] + g["wB"]]
                iB = iw.T.reshape(-1)[: cB * P].astype(np.int64)
                GA = table[:T0][iA].reshape(cA, P, ROW_F).transpose(1, 0, 2)
                GB = table[T0:][iB].reshape(cB, P, ROW_F).transpose(1, 0, 2)
                G = np.concatenate([GA, GB], axis=1)  # [128, C, 128]
                tt = np.concatenate(
                    [np.repeat(np.arange(tig), l0), np.repeat(np.arange(tig), l1)]
                )
                node_rows = (g["base"] + tt)[None, :] * P + np.arange(P)[:, None]
                a_d = adst_loc[m][node_rows]  # [128, C, 4]
                e = np.exp(leaky(G[:, :, 64:68] + a_d))  # [128, C, 4]
                s = np.zeros((P, tig, HEADS), np.float32)
                np.add.at(s, (slice(None), tt), e)
                rows0 = (g["base"] + np.arange(tig))[None, :] * P + np.arange(P)[:, None]
                s = s + e_self[rows0]  # [128, tig, 4]
                r = 1.0 / s
                msg = G[:, :, :64].reshape(P, C, HEADS, HID) * e[:, :, :, None]
                acc = np.zeros((P, tig, F_HID), np.float32)
                np.add.at(acc, (slice(None), tt), msg.reshape(P, C, F_HID))
                acc = acc + selfmsg.reshape(sp, F_HID)[rows0]
                acc = (
                    acc.reshape(P, tig, HEADS, HID) * r[:, :, :, None]
                ).reshape(P, tig, F_HID)
                for t in range(tig):
                    out[(g["base"] + t) * P + np.arange(P)] = acc[:, t]
            act = out + biases[layer][None, :]
            act = act * (1.0 / (1.0 + np.exp(-act)))
            acts[m] = act.astype(np.float32)

    for m in range(ncores):
        sel = percore["sel"][m].reshape(P, nt, gpc)
        a = acts[m].reshape(nt, P, F_HID)
        pooled = np.einsum("ptg,tpf->gf", sel, a)
        lg = pooled @ np.asarray(inputs["Wr"]) + np.asarray(inputs["br"])
        lg = np.maximum(lg, 0.0)
        mxv = lg.max(axis=1, keepdims=True)
        ls = lg - mxv - np.log(np.exp(lg - mxv).sum(axis=1, keepdims=True))
        out_logits[glob["graph_of"][m]] = ls
    return out_logits


# ======== kernel builder ========
"""Bass/Tile kernel for the distributed GAT (8 NeuronCores).

One SPMD program; per-core data arrives via in_maps. Cross-core: one
AllGather per layer (slice -> table).
"""

from contextlib import ExitStack

import concourse.bass as bass
import concourse.tile as tile
from concourse import bacc
from concourse import mybir
from concourse.library_config import mlp as mlp_lib
from concourse.masks import make_identity

F32 = mybir.dt.float32
BF16 = mybir.dt.bfloat16
I16 = mybir.dt.int16
AF = mybir.ActivationFunctionType
OP = mybir.AluOpType


def build_gat(S, n_cores=8):
    import os

    dbg_layers = int(os.environ.get("GAT_NLAYERS", "3"))
    sp = S["slice_pad"]
    nt = S["n_tiles"]
    Npad = S["Npad"]
    T0 = S["T0_ROWS"]
    groups = S["groups"]
    WA = S["WA"]
    WB = S["WB"]
    GPC = S["g_per_core"]
    NCLS = S["ncls"]
    FH = HEADS * HID  # 64
    LAYERS = dbg_layers
    in_dims = [S["f_in"], FH, FH]
    GCH = int(os.environ.get("GAT_GCH", "8"))

    nc = bacc.Bacc("TRN2", debug=False, num_devices=n_cores, num_swdge_queues=4)

    # ---------------- I/O ----------------
    xT_d = nc.dram_tensor("xT", [in_dims[0], sp], F32, kind="ExternalInput")
    idxA_d = nc.dram_tensor("idxA", [P, WA], I16, kind="ExternalInput")
    idxB_d = nc.dram_tensor("idxB", [P, WB], I16, kind="ExternalInput")
    sel_d = nc.dram_tensor("sel", [P, nt * GPC], F32, kind="ExternalInput")
    poison_d = nc.dram_tensor("poison", [P, nt * HEADS], F32, kind="ExternalInput")
    w_d = [
        nc.dram_tensor(f"w{l}aug", [in_dims[l], 72], F32, kind="ExternalInput")
        for l in range(3)
    ]
    bias_d = nc.dram_tensor("biases", [P, 3 * FH], F32, kind="ExternalInput")
    wr_d = nc.dram_tensor("wr", [FH, NCLS], F32, kind="ExternalInput")
    br_d = nc.dram_tensor("br", [P, NCLS], F32, kind="ExternalInput")
    out_d = nc.dram_tensor("out", [GPC, NCLS], F32, kind="ExternalOutput")

    slice_d = [nc.dram_tensor(f"slice{l}", [sp, P], BF16) for l in range(LAYERS)]
    table_d = [
        nc.dram_tensor(f"table{l}", [Npad, P], BF16, addr_space="Shared")
        for l in range(LAYERS)
    ]

    rg = [list(range(n_cores))]
    tigmax = max(g["tig"] for g in groups)

    with tile.TileContext(nc) as tc, ExitStack() as ctx:
        pers = ctx.enter_context(tc.tile_pool(name="pers", bufs=1))
        gpool = ctx.enter_context(tc.tile_pool(name="G", bufs=4))
        ltpool = ctx.enter_context(tc.tile_pool(name="lt", bufs=3))
        idxpool = ctx.enter_context(tc.tile_pool(name="idx", bufs=4))
        stat = ctx.enter_context(tc.tile_pool(name="stat", bufs=4))
        opool = ctx.enter_context(tc.tile_pool(name="oacc", bufs=3))
        rowp = ctx.enter_context(tc.tile_pool(name="row", bufs=3))
        psum = ctx.enter_context(tc.tile_pool(name="psum", bufs=2, space="PSUM"))
        psumT = ctx.enter_context(tc.tile_pool(name="psumT", bufs=2, space="PSUM"))

        # ---- persistent SBUF ----
        sel_sb = pers.tile([P, nt * GPC], F32)
        poison_sb = pers.tile([P, nt * HEADS], F32)
        w_sb = [
            pers.tile([in_dims[l], 72], F32, name=f"w{l}sb", tag=f"w{l}sb")
            for l in range(LAYERS)
        ]
        bias_sb = pers.tile([P, 3 * FH], F32)
        wr_sb = pers.tile([FH, NCLS], F32)
        br_sb = pers.tile([P, NCLS], F32)
        ident = pers.tile([P, P], F32)
        # double-buffered per-layer-parity state
        lslice = [pers.tile([P, nt * ROW_F], BF16, name=f"lslice{i}", tag=f"lslice{i}") for i in range(2)]
        adst_sb = [pers.tile([P, nt * HEADS], F32, name=f"adst{i}", tag=f"adst{i}") for i in range(2)]
        asrc_sb = [pers.tile([P, nt * HEADS], F32, name=f"asrc{i}", tag=f"asrc{i}") for i in range(2)]
        eself_sb = [pers.tile([P, nt * HEADS], F32, name=f"eself{i}", tag=f"eself{i}") for i in range(2)]
        selfmsg_sb = [pers.tile([P, nt * FH], F32, name=f"smsg{i}", tag=f"smsg{i}") for i in range(2)]
        out_sb = pers.tile([P, nt * FH], F32)  # aggregation output / act

        nc.sync.dma_start(sel_sb[:], sel_d[:])
        nc.sync.dma_start(poison_sb[:], poison_d[:])
        for l in range(LAYERS):
            nc.sync.dma_start(w_sb[l][:], w_d[l][:])
        nc.sync.dma_start(bias_sb[:], bias_d[:])
        nc.sync.dma_start(wr_sb[:], wr_d[:])
        nc.sync.dma_start(br_sb[:], br_d[:])
        make_identity(nc, ident[:])
        # table row cols 72:128 stay zero for the whole kernel
        for i in range(2):
            nc.vector.memset(
                lslice[i][:].rearrange("p (t f) -> p t f", f=ROW_F)[:, :, 72:], 0.0
            )

        nc.gpsimd.load_library(mlp_lib)

        _regs = {}
        qrr = [0]

        def nreg(v):
            if v not in _regs:
                _regs[v] = nc.gpsimd.to_reg(v)
            return _regs[v]

        def slice_build(l, c):
            """Build table-slice tile c for layer l (writes lslice, a_src/adst,
            slice_d). For l>0, reads act from out_sb (must be final for tile c).
            """
            par = l % 2
            if l == 0:
                lhsT = rowp.tile([in_dims[0], P], F32, tag="xchunk")
                nc.sync.dma_start(lhsT[:], xT_d[:, c * P : (c + 1) * P])
                lhsT_ap = lhsT[:]
            else:
                pT = psumT.tile([FH, P], F32)
                nc.tensor.transpose(
                    out=pT[:],
                    in_=out_sb[:, c * FH : (c + 1) * FH],
                    identity=ident[:],
                )
                aT = rowp.tile([FH, P], F32, tag="actT")
                nc.scalar.copy(aT[:], pT[:])
                lhsT_ap = aT[:]
            pR = psum.tile([P, 72], F32)
            nc.tensor.matmul(pR[:], lhsT=lhsT_ap, rhs=w_sb[l][:], start=True, stop=True)
            ls = lslice[par]
            nc.scalar.copy(ls[:, c * ROW_F : c * ROW_F + 64], pR[:, :64])
            nc.vector.tensor_tensor(
                out=ls[:, c * ROW_F + 64 : c * ROW_F + 72].bitcast(F32),
                in0=pR[:, 64:68],
                in1=poison_sb[:, c * HEADS : (c + 1) * HEADS],
                op=OP.add,
            )
            nc.scalar.copy(asrc_sb[par][:, c * HEADS : (c + 1) * HEADS], pR[:, 64:68])
            nc.scalar.copy(adst_sb[par][:, c * HEADS : (c + 1) * HEADS], pR[:, 68:72])
            nc.sync.dma_start(
                slice_d[l][c * P : (c + 1) * P, :],
                ls[:, c * ROW_F : (c + 1) * ROW_F],
            )

        def self_pre(l):
            """e_self and self-message for layer l from local state."""
            par = l % 2
            es = eself_sb[par]
            nc.vector.tensor_tensor(
                out=es[:], in0=asrc_sb[par][:], in1=adst_sb[par][:], op=OP.add
            )
            nc.vector.scalar_tensor_tensor(
                out=es[:], in0=es[:], scalar=NEG_SLOPE, in1=es[:],
                op0=OP.mult, op1=OP.max,
            )
            nc.scalar.activation(es[:], es[:], AF.Exp)
            sm3 = selfmsg_sb[par][:].rearrange("p (t h d) -> p t h d", h=HEADS, d=HID)
            hv = (
                lslice[par][:]
                .rearrange("p (t f) -> p t f", f=ROW_F)[:, :, :64]
                .rearrange("p t (h d) -> p t h d", d=HID)
            )
            ev = (
                es[:]
                .rearrange("p (t h) -> p t h", h=HEADS)
                .unsqueeze(3)
                .broadcast_to([P, nt, HEADS, HID])
            )
            nc.vector.tensor_tensor(out=sm3, in0=hv, in1=ev, op=OP.mult)

        def edge_group(l, g):
            par = l % 2
            tig, l0, l1 = g["tig"], g["L0"], g["L1"]
            cA, cB = tig * l0, tig * l1
            C = cA + cB
            base = g["base"]

            iA = idxpool.tile([P, g["wA"]], I16, tag="iA")
            nc.sync.dma_start(iA[:], idxA_d[:, g["offA"] : g["offA"] + g["wA"]])
            iB = idxpool.tile([P, g["wB"]], I16, tag="iB")
            nc.sync.dma_start(iB[:], idxB_d[:, g["offB"] : g["offB"] + g["wB"]])

            G = gpool.tile([P, C * P], BF16, tag="G")
            G3 = G[:].rearrange("p (c f) -> p c f", f=P)
            # chunk gathers to <=GCH*128 idxs; round-robin the 4 SWDGE queues
            for c0all, ccn, itile, tdsl in (
                (0, cA, iA, table_d[l][:T0, :]),
                (cA, cB, iB, table_d[l][T0:, :]),
            ):
                for k in range(0, ccn, GCH):
                    kc = min(GCH, ccn - k)
                    nc.gpsimd.dma_gather(
                        G3[:, c0all + k : c0all + k + kc, :],
                        tdsl,
                        itile[:, k * 8 : (k + kc) * 8],
                        kc * P,
                        nreg(kc * P),
                        P,
                        queue_num=qrr[0] % 4,
                    )
                    qrr[0] += 1

            lt = ltpool.tile([P, C * HEADS], F32, tag="lt")
            lt3 = lt[:].rearrange("p (c h) -> p c h", h=HEADS)
            # e = exp(leaky(a_src[gathered] + a_dst[local node]))
            adg = adst_sb[par][:, base * HEADS : (base + tig) * HEADS]
            for c0, cc, L in ((0, cA, l0), (cA, cB, l1)):
                adview = (
                    adg.rearrange("p (t h) -> p t h", h=HEADS)
                    .unsqueeze(2)
                    .broadcast_to([P, tig, L, HEADS])
                )
                nc.vector.tensor_tensor(
                    out=lt3[:, c0 : c0 + cc, :].rearrange("p (t l) h -> p t l h", l=L),
                    in0=G3[:, c0 : c0 + cc, 64:72]
                    .bitcast(F32)
                    .rearrange("p (t l) h -> p t l h", l=L),
                    in1=adview,
                    op=OP.add,
                )
            nc.vector.scalar_tensor_tensor(
                out=lt[:], in0=lt[:], scalar=NEG_SLOPE, in1=lt[:],
                op0=OP.mult, op1=OP.max,
            )
            nc.scalar.activation(lt[:], lt[:], AF.Exp)
            # s = segment sum + e_self ; r = 1/s
            s1 = stat.tile([P, tigmax * HEADS], F32, tag="s1")
            s2 = stat.tile([P, tigmax * HEADS], F32, tag="s2")
            nc.vector.reduce_sum(
                s1[:].rearrange("p (t h) -> p t h", h=HEADS)[:, :tig, :],
                lt3[:, :cA, :].rearrange("p (t l) h -> p t h l", l=l0),
                axis=mybir.AxisListType.X,
            )
            nc.vector.reduce_sum(
                s2[:].rearrange("p (t h) -> p t h", h=HEADS)[:, :tig, :],
                lt3[:, cA:, :].rearrange("p (t l) h -> p t h l", l=l1),
                axis=mybir.AxisListType.X,
            )
            nc.vector.tensor_tensor(
                out=s1[:, : tig * HEADS],
                in0=s1[:, : tig * HEADS],
                in1=s2[:, : tig * HEADS],
                op=OP.add,
            )
            nc.vector.tensor_tensor(
                out=s1[:, : tig * HEADS],
                in0=s1[:, : tig * HEADS],
                in1=eself_sb[par][:, base * HEADS : (base + tig) * HEADS],
                op=OP.add,
            )
            nc.vector.reciprocal(s1[:, : tig * HEADS], s1[:, : tig * HEADS])
            # msg = h * e (in place on G, bf16), all heads at once
            for c0, cc, L in ((0, cA, l0), (cA, cB, l1)):
                gv = G3[:, c0 : c0 + cc, :64].rearrange("p c (h d) -> p c h d", d=HID)
                evw = (
                    lt3[:, c0 : c0 + cc, :]
                    .unsqueeze(3)
                    .broadcast_to([P, cc, HEADS, HID])
                )
                nc.vector.tensor_tensor(out=gv, in0=gv, in1=evw, op=OP.mult)
            # out = segment sum of messages + selfmsg, then * r
            oA = opool.tile([P, tigmax * FH], F32, tag="oA")
            oB = opool.tile([P, tigmax * FH], F32, tag="oB")
            nc.vector.reduce_sum(
                oA[:].rearrange("p (t f) -> p t f", f=FH)[:, :tig, :],
                G3[:, :cA, :64].rearrange("p (t l) f -> p t f l", l=l0),
                axis=mybir.AxisListType.X,
            )
            nc.vector.reduce_sum(
                oB[:].rearrange("p (t f) -> p t f", f=FH)[:, :tig, :],
                G3[:, cA:, :64].rearrange("p (t l) f -> p t f l", l=l1),
                axis=mybir.AxisListType.X,
            )
            osl = out_sb[:, base * FH : (base + tig) * FH]
            nc.vector.tensor_tensor(
                out=osl, in0=oA[:, : tig * FH], in1=oB[:, : tig * FH], op=OP.add
            )
            nc.vector.tensor_tensor(
                out=osl,
                in0=osl,
                in1=selfmsg_sb[par][:, base * FH : (base + tig) * FH],
                op=OP.add,
            )
            rv = (
                s1[:, : tig * HEADS]
                .rearrange("p (t h) -> p t h", h=HEADS)
                .unsqueeze(3)
                .broadcast_to([P, tig, HEADS, HID])
            )
            o4 = osl.rearrange("p (t h d) -> p t h d", h=HEADS, d=HID)
            nc.vector.tensor_tensor(out=o4, in0=o4, in1=rv, op=OP.mult)
            # bias + silu
            blg = (
                bias_sb[:, l * FH : (l + 1) * FH]
                .unsqueeze(1)
                .broadcast_to([P, tig, FH])
            )
            o3 = osl.rearrange("p (t f) -> p t f", f=FH)
            nc.vector.tensor_tensor(out=o3, in0=o3, in1=blg, op=OP.add)
            sgg = opool.tile([P, tigmax * FH], F32, tag="sgg")
            nc.scalar.activation(sgg[:, : tig * FH], osl, AF.Sigmoid)
            nc.vector.tensor_tensor(
                out=osl, in0=osl, in1=sgg[:, : tig * FH], op=OP.mult
            )

        # ================= main layer loop =================
        for layer in range(LAYERS):
            if layer == 0:
                for c in range(nt):
                    slice_build(0, c)
            nc.gpsimd.collective_compute(
                "AllGather",
                mybir.AluOpType.bypass,
                replica_groups=rg,
                ins=[slice_d[layer].ap().opt()],
                outs=[table_d[layer].ap().opt()],
            )
            self_pre(layer)
            for g in groups:
                edge_group(layer, g)
                if layer + 1 < LAYERS:
                    for c in range(g["base"], g["base"] + g["tig"]):
                        slice_build(layer + 1, c)

        # ================= pooling + classifier =================
        pP = psum.tile([GPC, FH], F32, tag="pool", bufs=1)
        for t in range(nt):
            nc.tensor.matmul(
                pP[:],
                lhsT=sel_sb[:, t * GPC : (t + 1) * GPC],
                rhs=out_sb[:, t * FH : (t + 1) * FH].rearrange("p f -> p f"),
                start=(t == 0),
                stop=(t == nt - 1),
            )
        pooled = rowp.tile([GPC, FH], F32, tag="pooled")
        nc.vector.tensor_copy(pooled[:], pP[:])
        pTpsum = psumT.tile([FH, GPC], F32, tag="poolT", bufs=1)
        nc.tensor.transpose(out=pTpsum[:], in_=pooled[:], identity=ident[:GPC, :GPC])
        pooledT = rowp.tile([FH, GPC], F32, tag="pooledT")
        nc.vector.tensor_copy(pooledT[:], pTpsum[:])
        lgP = psum.tile([GPC, NCLS], F32, tag="lg", bufs=1)
        nc.tensor.matmul(lgP[:], lhsT=pooledT[:], rhs=wr_sb[:], start=True, stop=True)
        lg = rowp.tile([GPC, NCLS], F32, tag="lgs")
        nc.vector.tensor_tensor(out=lg[:], in0=lgP[:], in1=br_sb[:GPC, :], op=OP.add)
        nc.scalar.activation(lg[:], lg[:], AF.Relu)
        # log softmax
        mx = stat.tile([GPC, 1], F32, tag="mx")
        nc.vector.reduce_max(mx[:], lg[:], axis=mybir.AxisListType.X)
        nc.vector.tensor_tensor(
            out=lg[:], in0=lg[:], in1=mx[:].broadcast_to([GPC, NCLS]), op=OP.subtract
        )
        ex = rowp.tile([GPC, NCLS], F32, tag="ex")
        nc.scalar.activation(ex[:], lg[:], AF.Exp)
        sm = stat.tile([GPC, 1], F32, tag="sm")
        nc.vector.reduce_sum(sm[:], ex[:], axis=mybir.AxisListType.X)
        lnm = stat.tile([GPC, 1], F32, tag="lnm")
        nc.scalar.activation(lnm[:], sm[:], AF.Ln)
        nc.vector.tensor_tensor(
            out=lg[:], in0=lg[:], in1=lnm[:].broadcast_to([GPC, NCLS]), op=OP.subtract
        )
        nc.sync.dma_start(out_d[:], lg[:])

    nc.compile()
    return nc


# ======== kernel(**inputs) entry point ========

import os


_NCORES = 8
_NG = 128


def kernel(**inputs) -> np.ndarray:
    x = np.asarray(inputs["x"], np.float32)
    ei = np.asarray(inputs["edge_index"])
    batch = np.asarray(inputs["batch"])

    struct, percore, glob = build_plan(
        x, ei, batch, ng=_NG, ncores=_NCORES,
        ccap=int(os.environ.get("GAT_CCAP", "64")),
        tigmax=int(os.environ.get("GAT_TIGMAX", "8")),
    )
    struct["ncls"] = int(np.asarray(inputs["Wr"]).shape[1])

    nc = build_gat(struct, n_cores=_NCORES)
    in_maps = make_inmaps(inputs, struct, percore)

    from concourse.bass_utils import run_bass_kernel_spmd

    trace = os.environ.get("GAT_TRACE", "0") == "1"
    res = run_bass_kernel_spmd(
        nc,
        in_maps,
        core_ids=list(range(_NCORES)),
        trace=trace,
    )
    if res.exec_time_ns is not None:
        print(f"HW exec time: {res.exec_time_ns} ns", flush=True)
        if res.mean_exec_time_ns is not None:
            print(f"HW exec time (mean): {res.mean_exec_time_ns:.0f} ns", flush=True)
        if res.instructions_and_trace is not None:
            print(f"trace: {res.instructions_and_trace[1]}", flush=True)

    out = np.zeros((_NG, struct["ncls"]), np.float32)
    for m in range(_NCORES):
        out[glob["graph_of"][m]] = res.results[m]["out"]
    return out
